# revision 2
# baseline (speedup 1.0000x reference)
"""RNN-T Joint network kernel for 8x Trainium2 NeuronCores.

logits[b,t,u,v] = enc_out[b,t,:] @ W[v,:512] + pred_out[b,u,:] @ W[v,512:] + b[v]

Sharding: data-parallel over (B=4) x (T split in 2) -> 8 shards.
Core i handles b = i//2, t in [128*(i%2), 128*(i%2)+128).
Each core computes a contiguous (128, 64, 2048) f32 output slab (64 MB).

Per-core plan (layout: t on partitions, v on free dim):
  1. Load W (2048,1024) contiguous; PE-transpose 128x128 blocks to get
     W^T with the contraction dim d on partitions.  Same for enc/pred slices.
  2. enc_proj (128t, 2048v) and pred_proj+bias (64u, 2048v) via fp32 matmuls.
  3. For each u: broadcast pred_proj[u,:] to 128 partitions with a K=1
     ones-matmul (float32r: 1 cyc/row) into PSUM, DVE-add enc_proj + PSUM
     -> SBUF, then 1 MB DMA store (8 KB contiguous per partition).
"""

import numpy as np

B, T, U = 4, 256, 64
D_ENC, D_PRED, VOCAB = 512, 512, 2048
D = D_ENC + D_PRED
TT = 128  # t rows per core
N_CORES = 8

_cache = {}


def _build():
    import concourse.bacc as bacc
    import concourse.mybir as mybir
    from concourse.tile import TileContext

    f32 = mybir.dt.float32
    f32r = mybir.dt.float32r

    nc = bacc.Bacc("TRN2", target_bir_lowering=False, debug=False, num_devices=N_CORES)
    enc_d = nc.dram_tensor("enc", (TT, D_ENC), f32, kind="ExternalInput")
    # identity | bias | 1.0  packed into one tensor -> one DMA -> one sem lane
    const_d = nc.dram_tensor("consts", (128, 128 + VOCAB + 1), f32, kind="ExternalInput")
    onehot_d = nc.dram_tensor("onehotr", (U, U * 128), f32r, kind="ExternalInput")
    pred_d = nc.dram_tensor("pred", (U, D_PRED), f32, kind="ExternalInput")
    w_d = nc.dram_tensor("w", (VOCAB, D), f32, kind="ExternalInput")
    out_d = nc.dram_tensor("out", (TT, U, VOCAB), f32, kind="ExternalOutput")

    NV = VOCAB // 512          # 4 chunks of 512 along v
    KD = D // 128              # 8 k-tiles along d
    KE = D_ENC // 128          # 4 enc k-tiles

    with TileContext(nc) as tc:
        with (
            tc.tile_pool(name="const", bufs=1) as const,
            tc.tile_pool(name="persist", bufs=1) as persist,
            tc.tile_pool(name="outp", bufs=4) as outp,
        ):
            # consts layout: [:, :128] identity; [0, 128:128+V] bias; [0, -1] 1.0
            consts = const.tile([128, 128 + VOCAB + 1], f32)
            nc.sync.dma_start(out=consts, in_=const_d[:])
            ident = consts[:, 0:128]
            bias_sb = consts[0:1, 128:128 + VOCAB]
            ones = consts[0:1, 128 + VOCAB:]
            # onehot[u] block: (64, 128), row u all-ones — lhsT that broadcasts
            # pred_proj row u across 128 output partitions (fp32r, 1 cyc/row)
            onehot = const.tile([U, U * 128], f32r)
            nc.sync.dma_start(out=onehot, in_=onehot_d[:])

            w_t = persist.tile([128, KD * VOCAB], f32)      # W^T: [d_tile k][128d, 2048v]
            enc_proj = persist.tile([128, VOCAB], f32)
            pred_proj = persist.tile([U, VOCAB], f32r)
            enc_t = persist.tile([128, KE * TT], f32)       # enc^T k-tiles
            pred_t = persist.tile([128, KE * U], f32)       # pred^T k-tiles
            with (
                tc.tile_pool(name="loads", bufs=3) as loads,
                tc.tile_pool(name="ps_a", bufs=2, space="PSUM") as ps_a,
            ):
                # PE pre-consumes each const DMA once (1 wait per inst) so no
                # later instruction needs >2 sync-wait commands (ISA limit).
                ps_dummy = ps_a.tile([128, 512], f32, tag="ps_dummy")
                nc.tensor.transpose(ps_dummy[:, :128], ident, ident)
                nc.tensor.matmul(
                    ps_dummy,
                    lhsT=onehot[:, 0:128],
                    rhs=onehot[:, 0:512],
                    start=True,
                    stop=True,
                )
                # ---- W^T: load 16 row-tiles of W, transpose 128x128 blocks
                for vt in range(VOCAB // 128):
                    w_raw = loads.tile([128, D], f32)
                    nc.sync.dma_start(out=w_raw, in_=w_d[vt * 128:(vt + 1) * 128, :])
                    for k in range(KD):
                        ps_tr = ps_a.tile([128, 128], f32)
                        nc.tensor.transpose(ps_tr, w_raw[:, k * 128:(k + 1) * 128], ident)
                        nc.scalar.copy(
                            out=w_t[:, k * VOCAB + vt * 128: k * VOCAB + vt * 128 + 128],
                            in_=ps_tr,
                        )
                # ---- enc^T
                enc_sb = loads.tile([128, D_ENC], f32, tag="enc_sb")
                nc.sync.dma_start(out=enc_sb, in_=enc_d[:])
                for k in range(KE):
                    ps_tr = ps_a.tile([128, 128], f32)
                    nc.tensor.transpose(ps_tr, enc_sb[:, k * 128:(k + 1) * 128], ident)
                    nc.scalar.copy(out=enc_t[:, k * TT:(k + 1) * TT], in_=ps_tr)
                # ---- pred^T
                pred_sb = loads.tile([U, D_PRED], f32, tag="pred_sb")
                nc.sync.dma_start(out=pred_sb, in_=pred_d[:])
                for k in range(KE):
                    ps_tr = ps_a.tile([128, 128], f32)
                    nc.tensor.transpose(
                        ps_tr[:, :U], pred_sb[:, k * 128:(k + 1) * 128], ident[:U, :U]
                    )
                    nc.scalar.copy(out=pred_t[:, k * U:(k + 1) * U], in_=ps_tr[:, :U])

                # ---- enc_proj (128t, 2048v)
                for c in range(NV):
                    ps = ps_a.tile([128, 512], f32, tag="ps_proj")
                    for k in range(KE):
                        nc.tensor.matmul(
                            ps,
                            lhsT=enc_t[:, k * TT:(k + 1) * TT],
                            rhs=w_t[:, k * VOCAB + c * 512: k * VOCAB + (c + 1) * 512],
                            start=(k == 0),
                            stop=(k == KE - 1),
                        )
                    nc.scalar.copy(out=enc_proj[:, c * 512:(c + 1) * 512], in_=ps)
                # ---- pred_proj (64u, 2048v) + bias
                for c in range(NV):
                    ps = ps_a.tile([128, 512], f32, tag="ps_proj")
                    for k in range(KE):
                        kd = KE + k  # W_pred half
                        nc.tensor.matmul(
                            ps[:U],
                            lhsT=pred_t[:, k * U:(k + 1) * U],
                            rhs=w_t[:, kd * VOCAB + c * 512: kd * VOCAB + (c + 1) * 512],
                            start=(k == 0),
                            stop=False,
                        )
                    nc.tensor.matmul(
                        ps[:U],
                        lhsT=ones.broadcast_to((1, U)),
                        rhs=bias_sb[:, c * 512:(c + 1) * 512],
                        start=False,
                        stop=True,
                    )
                    nc.scalar.copy(out=pred_proj[:, c * 512:(c + 1) * 512], in_=ps[:U])

            # ---- main loop: one (128, 2048) output tile per u
            with tc.tile_pool(name="ps_b", bufs=2, space="PSUM") as ps_b:
                onehot_r = onehot
                pred_r = pred_proj
                for u in range(U):
                    ps = ps_b.tile([128, VOCAB], f32)
                    for c in range(NV):
                        nc.tensor.matmul(
                            ps[:, c * 512:(c + 1) * 512],
                            lhsT=onehot_r[:, u * 128:(u + 1) * 128],
                            rhs=pred_r[:, c * 512:(c + 1) * 512],
                            start=True,
                            stop=True,
                        )
                    o = outp.tile([128, VOCAB], f32)
                    nc.vector.tensor_add(o, enc_proj, ps)
                    nc.sync.dma_start(out=out_d[:, u, :], in_=o)

    nc.compile()
    return nc


def _make_in_maps(enc_out, pred_out, W, b):
    w_c = np.ascontiguousarray(W.astype(np.float32))
    consts = np.zeros((128, 128 + VOCAB + 1), dtype=np.float32)
    consts[:128, :128] = np.eye(128, dtype=np.float32)
    consts[0, 128:128 + VOCAB] = b.astype(np.float32)
    consts[0, -1] = 1.0
    onehot = np.zeros((U, U * 128), dtype=np.float32)
    for u in range(U):
        onehot[u, u * 128:(u + 1) * 128] = 1.0
    in_maps = []
    for i in range(N_CORES):
        bi, th = i // 2, i % 2
        in_maps.append({
            "enc": np.ascontiguousarray(enc_out[bi, th * TT:(th + 1) * TT, :].astype(np.float32)),
            "pred": np.ascontiguousarray(pred_out[bi].astype(np.float32)),
            "w": w_c,
            "consts": consts,
            "onehotr": onehot,
        })
    return in_maps


def kernel(enc_out, pred_out, W, b):
    import os

    from concourse.bass_utils import run_bass_kernel_spmd

    if "nc" not in _cache:
        _cache["nc"] = _build()
    nc = _cache["nc"]
    trace = bool(os.environ.get("KJN_TRACE"))

    in_maps = _make_in_maps(enc_out, pred_out, W, b)

    kw = {}
    if trace:
        kw = dict(trace=True, trace_cores=[0], stitch_traces=False)
    res = run_bass_kernel_spmd(nc, in_maps, core_ids=list(range(N_CORES)), **kw)
    if trace:
        print(f"HW exec time: {res.exec_time_ns} ns")
        print(f"trace: {res.instructions_and_trace[1] if res.instructions_and_trace else None}")
        print(f"profile_json: {res.profile_json}")
    out = np.empty((B, T, U, VOCAB), dtype=np.float32)
    for i in range(N_CORES):
        bi, th = i // 2, i % 2
        out[bi, th * TT:(th + 1) * TT] = res.results[i]["out"]
    return out



# revision 7
# speedup vs baseline: 2.8350x; 2.8350x over previous
"""RNN-T Joint network kernel for 8x Trainium2 NeuronCores.

logits[b,t,u,v] = enc_out[b,t,:] @ W[v,:512] + pred_out[b,u,:] @ W[v,512:] + b[v]

Sharding: data-parallel over (B=4) x (T split in 2) -> 8 shards.
Core i handles b = i//2, t in [128*(i%2), 128*(i%2)+128).
Each core computes a contiguous (128, 64, 2048) f32 output slab (64 MB).

Per-core plan (layout: t on partitions, v on free dim):
  1. Load W (2048,1024) contiguous; PE-transpose 128x128 blocks to get
     W^T with the contraction dim d on partitions.  Same for enc/pred slices.
  2. enc_proj (128t, 2048v) and pred_proj+bias (64u, 2048v) via fp32 matmuls.
  3. For each u: broadcast pred_proj[u,:] to 128 partitions with a K=1
     ones-matmul (float32r: 1 cyc/row) into PSUM, DVE-add enc_proj + PSUM
     -> SBUF, then 1 MB DMA store (8 KB contiguous per partition).
"""

import numpy as np

B, T, U = 4, 256, 64
D_ENC, D_PRED, VOCAB = 512, 512, 2048
D = D_ENC + D_PRED
TT = 128  # t rows per core
N_CORES = 8

_cache = {}


def _build(reps=1, main_only=False):
    import concourse.bacc as bacc
    import concourse.mybir as mybir
    from concourse.tile import TileContext

    f32 = mybir.dt.float32
    f32r = mybir.dt.float32r

    nc = bacc.Bacc("TRN2", target_bir_lowering=False, debug=False, num_devices=N_CORES)
    if reps != 1:
        # bench-only: unused input whose shape encodes the bench config, so
        # the neuron compile cache (which doesn't hash the BIR) can't collide
        nc.dram_tensor("rep_marker", (1 + int(main_only), reps), f32,
                       kind="ExternalInput")
    enc_d = nc.dram_tensor("enc", (TT, D_ENC), f32, kind="ExternalInput")
    # identity | bias | 1.0  packed into one tensor -> one DMA -> one sem lane
    const_d = nc.dram_tensor("consts", (128, 128 + VOCAB + 1), f32, kind="ExternalInput")
    onehot_d = nc.dram_tensor("onehotr", (U, U * 128), f32r, kind="ExternalInput")
    pred_d = nc.dram_tensor("pred", (U, D_PRED), f32, kind="ExternalInput")
    w_d = nc.dram_tensor("w", (VOCAB, D), f32, kind="ExternalInput")
    out_d = nc.dram_tensor("out", (TT, U, VOCAB), f32, kind="ExternalOutput")

    NV = VOCAB // 512          # 4 chunks of 512 along v
    KD = D // 128              # 8 k-tiles along d
    KE = D_ENC // 128          # 4 enc k-tiles

    with TileContext(nc) as tc:
        with (
            tc.tile_pool(name="const", bufs=1) as const,
            tc.tile_pool(name="persist", bufs=1) as persist,
            tc.tile_pool(name="outp", bufs=4) as outp,
        ):
            # consts layout: [:, :128] identity; [0, 128:128+V] bias; [0, -1] 1.0
            consts = const.tile([128, 128 + VOCAB + 1], f32)
            nc.sync.dma_start(out=consts, in_=const_d[:])
            ident = consts[:, 0:128]
            bias_sb = consts[0:1, 128:128 + VOCAB]
            ones = consts[0:1, 128 + VOCAB:]
            # onehot[u] block: (64, 128), row u all-ones — lhsT that broadcasts
            # pred_proj row u across 128 output partitions (fp32r, 1 cyc/row)
            onehot = const.tile([U, U * 128], f32r)
            nc.sync.dma_start(out=onehot, in_=onehot_d[:])

            def _prologue():
                w_t = persist.tile([128, KD * VOCAB], f32, tag="w_t")  # W^T
                enc_proj = persist.tile([128, VOCAB], f32, tag="enc_proj")
                pred_proj = persist.tile([U, VOCAB], f32r, tag="pred_proj")
                enc_t = persist.tile([128, KE * TT], f32, tag="enc_t")  # enc^T
                pred_t = persist.tile([128, KE * U], f32, tag="pred_t")  # pred^T
                with (
                    tc.tile_pool(name="loads", bufs=3) as loads,
                    tc.tile_pool(name="ps_a", bufs=2, space="PSUM") as ps_a,
                ):
                    # PE pre-consumes each const DMA once (1 wait per inst) so no
                    # later instruction needs >2 sync-wait commands (ISA limit).
                    ps_dummy = ps_a.tile([128, 512], f32, tag="ps_dummy")
                    nc.tensor.transpose(ps_dummy[:, :128], ident, ident)
                    nc.tensor.matmul(
                        ps_dummy,
                        lhsT=onehot[:, 0:128],
                        rhs=onehot[:, 0:512],
                        start=True,
                        stop=True,
                    )
                    # ---- W^T: load 16 row-tiles of W, transpose 128x128 blocks
                    for vt in range(VOCAB // 128):
                        w_raw = loads.tile([128, D], f32)
                        nc.sync.dma_start(out=w_raw, in_=w_d[vt * 128:(vt + 1) * 128, :])
                        for k in range(KD):
                            ps_tr = ps_a.tile([128, 128], f32)
                            nc.tensor.transpose(ps_tr, w_raw[:, k * 128:(k + 1) * 128], ident)
                            nc.scalar.copy(
                                out=w_t[:, k * VOCAB + vt * 128: k * VOCAB + vt * 128 + 128],
                                in_=ps_tr,
                            )
                    # ---- enc^T
                    enc_sb = loads.tile([128, D_ENC], f32, tag="enc_sb")
                    nc.sync.dma_start(out=enc_sb, in_=enc_d[:])
                    for k in range(KE):
                        ps_tr = ps_a.tile([128, 128], f32)
                        nc.tensor.transpose(ps_tr, enc_sb[:, k * 128:(k + 1) * 128], ident)
                        nc.scalar.copy(out=enc_t[:, k * TT:(k + 1) * TT], in_=ps_tr)
                    # ---- pred^T
                    pred_sb = loads.tile([U, D_PRED], f32, tag="pred_sb")
                    nc.sync.dma_start(out=pred_sb, in_=pred_d[:])
                    for k in range(KE):
                        ps_tr = ps_a.tile([128, 128], f32)
                        nc.tensor.transpose(
                            ps_tr[:, :U], pred_sb[:, k * 128:(k + 1) * 128], ident[:U, :U]
                        )
                        nc.scalar.copy(out=pred_t[:, k * U:(k + 1) * U], in_=ps_tr[:, :U])

                    # ---- enc_proj (128t, 2048v)
                    for c in range(NV):
                        ps = ps_a.tile([128, 512], f32, tag="ps_proj")
                        for k in range(KE):
                            nc.tensor.matmul(
                                ps,
                                lhsT=enc_t[:, k * TT:(k + 1) * TT],
                                rhs=w_t[:, k * VOCAB + c * 512: k * VOCAB + (c + 1) * 512],
                                start=(k == 0),
                                stop=(k == KE - 1),
                            )
                        nc.scalar.copy(out=enc_proj[:, c * 512:(c + 1) * 512], in_=ps)
                    # ---- pred_proj (64u, 2048v) + bias
                    for c in range(NV):
                        ps = ps_a.tile([128, 512], f32, tag="ps_proj")
                        for k in range(KE):
                            kd = KE + k  # W_pred half
                            nc.tensor.matmul(
                                ps[:U],
                                lhsT=pred_t[:, k * U:(k + 1) * U],
                                rhs=w_t[:, kd * VOCAB + c * 512: kd * VOCAB + (c + 1) * 512],
                                start=(k == 0),
                                stop=False,
                            )
                        nc.tensor.matmul(
                            ps[:U],
                            lhsT=ones.broadcast_to((1, U)),
                            rhs=bias_sb[:, c * 512:(c + 1) * 512],
                            start=False,
                            stop=True,
                        )
                        nc.scalar.copy(out=pred_proj[:, c * 512:(c + 1) * 512], in_=ps[:U])
                return enc_proj, pred_proj

            def _main(enc_proj, pred_proj):
                # ---- main loop: one (128, 2048) output tile per u
                with tc.tile_pool(name="ps_b", bufs=2, space="PSUM") as ps_b:
                    for u in range(U):
                        ps = ps_b.tile([128, VOCAB], f32)
                        for c in range(NV):
                            nc.tensor.matmul(
                                ps[:, c * 512:(c + 1) * 512],
                                lhsT=onehot[:, u * 128:(u + 1) * 128],
                                rhs=pred_proj[:, c * 512:(c + 1) * 512],
                                start=True,
                                stop=True,
                            )
                        o = outp.tile([128, VOCAB], f32)
                        nc.vector.tensor_add(o, enc_proj, ps)
                        nc.sync.dma_start(out=out_d[:, u, :], in_=o)

            if main_only:
                h = _prologue()
                for _rep in range(reps):
                    _main(*h)
            else:
                for _rep in range(reps):
                    _main(*_prologue())
    nc.compile()
    return nc


def _make_in_maps(enc_out, pred_out, W, b):
    w_c = np.ascontiguousarray(W.astype(np.float32))
    consts = np.zeros((128, 128 + VOCAB + 1), dtype=np.float32)
    consts[:128, :128] = np.eye(128, dtype=np.float32)
    consts[0, 128:128 + VOCAB] = b.astype(np.float32)
    consts[0, -1] = 1.0
    onehot = np.zeros((U, U * 128), dtype=np.float32)
    for u in range(U):
        onehot[u, u * 128:(u + 1) * 128] = 1.0
    in_maps = []
    for i in range(N_CORES):
        bi, th = i // 2, i % 2
        in_maps.append({
            "enc": np.ascontiguousarray(enc_out[bi, th * TT:(th + 1) * TT, :].astype(np.float32)),
            "pred": np.ascontiguousarray(pred_out[bi].astype(np.float32)),
            "w": w_c,
            "consts": consts,
            "onehotr": onehot,
        })
    return in_maps


def kernel(enc_out, pred_out, W, b):
    import os

    from concourse.bass_utils import run_bass_kernel_spmd

    if "nc" not in _cache:
        _cache["nc"] = _build()
    nc = _cache["nc"]
    trace = bool(os.environ.get("KJN_TRACE"))

    in_maps = _make_in_maps(enc_out, pred_out, W, b)

    kw = {}
    if trace:
        kw = dict(trace=True, trace_cores=[0], stitch_traces=False)
    res = run_bass_kernel_spmd(nc, in_maps, core_ids=list(range(N_CORES)), **kw)
    if trace:
        print(f"HW exec time: {res.exec_time_ns} ns")
        print(f"trace: {res.instructions_and_trace[1] if res.instructions_and_trace else None}")
        print(f"profile_json: {res.profile_json}")
    out = np.empty((B, T, U, VOCAB), dtype=np.float32)
    for i in range(N_CORES):
        bi, th = i // 2, i % 2
        out[bi, th * TT:(th + 1) * TT] = res.results[i]["out"]
    return out
